# revision 4
# baseline (speedup 1.0000x reference)
"""Trainium2 Bass kernel for nn_BioSimulator (phosphene pooling model).

Math: gauss separates as gy[n,h]*gx[n,w], so out = poly(GY^T @ GXB) with
K = 1024 electrodes (8 chunks of 128 partitions) accumulated in PSUM over
8 bf16 matmuls (single PE pass vs the fp32 LOW/HIGH double pass).

Key structure (raw bacc, no TileContext; ~20.1us HW vs 31us baseline):
- Reduced complex algebra for z = ab(ew-1)/(b-a*ew): with er2 = er^2,
  num_r = -a*er2 + (a+b)*ewr - b, num_i = (b-a)*ewi,
  den = a^2*er2 - 2ab*ewr + b^2; ab / ab(b-a) factors fold into constants.
- sin/cos as packed [sin|cos] Estrin ladders on [128, 16] tiles; real/imag
  pairs packed likewise ([num_r|ewi] etc.) to halve DVE op count.
- Electrode centers decoupled from sigma: dxr/dyr = pixel - center are
  single [128, 512]/[128, 1024] broadcast tensor ops (stride-0 APs) that
  run while the sigma tail (packed ln/exp sqrt of [r|sbase]) resolves;
  their squares run on the otherwise idle ACT engine (Square), one y-half
  on gpsimd (delayed past dyrA: DVE+GPSIMD share SBUF ports, running both
  concurrently halves each one's bandwidth).
- sq = dxr2 * rs2 with rs2 = 1/(2 sigma_px^2) on broadcast pair views;
  exp on ACT in [128, 384] pair batches straight to bf16.
- The ACT table load's ~2us data fetch shares a DMA engine with one input
  queue share; it is gated behind the first input-DMA sem increment so the
  input's descriptors win the queue (fixes a 1.5-2us straggler).
- The framework's const-AP memsets and the trailing start barrier are
  stripped from the entry block (activation biases come from input-image
  constant columns); no kernel sem-clear epilogue at all - the NEFF
  postamble's full-range semaphore reset makes both redundant.
- DVE must not feed tensor_scalar straight from PSUM (hangs the NEFF);
  the quartic tail copies PSUM through SBUF, with e1/e2p on ACT in
  parallel.

Sharding: 2x4 grid over the output; every core evaluates all 1024
electrodes for its [128, 64] slice (no collectives); host stitches.
"""

import numpy as np

GRID = 32
OUT = 256
FOV = 30.0
N_CORES = 8
NCHUNK = 8

K_, A_, B_ = 17.3, 0.75, 120.0
SLOPE, HALF, RHEO = 19152642.5, 1.057e-07, 2.39e-05
FREQ, PW, R2S = 300.0, 0.00017, 0.5
DEG2PIX = OUT / (2.0 * FOV)
DEG2RAD = float(np.pi / 180.0)
INVK = 1.0 / K_
AB = A_ * B_
BMA = B_ - A_
CMA = 1.0 / (K_ * BMA)
SLP = SLOPE * PW * FREQ
ESH = float(np.exp(SLOPE * HALF))
SQRT2 = float(np.sqrt(2.0))
C_VX = AB * DEG2PIX
C_VY = AB * BMA * DEG2PIX
CSG = R2S * DEG2PIX

SIN_C = (0.999999993645295, -0.1666663839873324, 0.008331410967920568,
         -0.00019428598847529545)
COS_C = (0.9999999430059742, -0.49999746415333846, 0.041649415317051235,
         -0.0013518287615003882)
SIN_CK = tuple(c * INVK for c in SIN_C)

STRIP_MEMSETS = True
STRIP_START_BARRIER = True
BF16_BULK = True
ZERO_BIAS_COLS = True
E1_ON_DVE = True

# packed input layout:
# [stim(8) | pp(13) | gxe(8) | gye(8) | gxer(8) | gyer(8) | cn(20) |
#  pxs(64) | pys(128)]
C_STIM, C_PP, C_GXE, C_GYE, C_GXER, C_GYER = 0, 8, 21, 29, 37, 45
C_CN = 53        # [-RHEO, CMA*AB, 0.0, 1.0, KV(16)]
C_KV = C_CN + 4
C_PXS, C_PYS, C_END = 73, 137, 265

_CACHE: dict = {}


def _host_constants():
    if "consts" in _CACHE:
        return _CACHE["consts"]
    xc = np.linspace(-15.0, 15.0, GRID, dtype=np.float32)
    gx, gy = np.meshgrid(xc, xc, indexing="xy")
    gxe = gx.reshape(-1).astype(np.float32).reshape(NCHUNK, 128).T.copy()
    gye = gy.reshape(-1).astype(np.float32).reshape(NCHUNK, 128).T.copy()
    xs = np.linspace(-FOV, FOV, OUT, dtype=np.float32)
    _CACHE["consts"] = (gxe, gye, xs)
    return _CACHE["consts"]


def _build_nc(self_waits=False):
    """Build the SPMD raw-bacc program (same program on all 8 cores)."""
    key = ("nc", self_waits)
    if key in _CACHE:
        return _CACHE[key]

    import concourse.bacc as bacc
    import concourse.mybir as mybir
    from concourse.bass import AP

    f32 = mybir.dt.float32
    bf16 = mybir.dt.bfloat16
    AF = mybir.ActivationFunctionType
    OP = mybir.AluOpType

    class _Bacc(bacc.Bacc):
        # exp/ln/square/copy/relu/identity only resolvable from the
        # natural_log_exp_and_others set -> one ACT table load.
        def insert_act_table_loads(self):
            from concourse.hw_specs import get_activation_tables
            from concourse import bacc as _bacc_mod

            has_activation = any(
                isinstance(i, mybir.InstActivation)
                for b in self.main_func.blocks
                for i in b.instructions
            )
            if not has_activation:
                return
            tabs = get_activation_tables(self.m.arch)
            pref = "natural_log_exp_and_others"
            ours = {AF.Exp, AF.Ln, AF.Square, AF.Copy, AF.Relu, AF.Identity}
            tables = [
                (k, (v if k == pref else (v - ours))) for k, v in tabs.items()
            ]
            _bacc_mod._bass_rust.insert_act_table_loads(self, tables)
            # The table's ~2us data fetch shares a DMA engine with one
            # input-DMA queue share; unless gated it runs first and delays
            # the input by that much.
            gate = getattr(self, "_act_gate", None)
            if gate is not None:
                import concourse.bass as _bass_mod
                for b in self.main_func.blocks:
                    for i in b.instructions:
                        if isinstance(i, mybir.InstLoadActFuncSet):
                            _bass_mod.BassInstruction(i)._wait_ge(*gate)

    nc = _Bacc(None, detect_race_conditions=self_waits)
    if STRIP_MEMSETS:
        # Strip the const-AP memsets: we never use framework const APs (all
        # activation biases are APs into the input image), and the gpsimd
        # memsets delay the start barrier by ~0.9us.
        blk0 = nc.main_func.blocks[0]
        blk0.instructions = [
            i for i in blk0.instructions
            if not isinstance(i, mybir.InstMemset)
        ]
    if STRIP_START_BARRIER:
        # Bass.__init__'s trailing all_engine_barrier only fences the const
        # memsets (now stripped); every engine's first kernel op is gated on
        # the input DMA semaphore anyway.
        blk0 = nc.main_func.blocks[0]
        blk0.instructions = [
            i for i in blk0.instructions
            if not isinstance(i, (mybir.InstDrain, mybir.InstEventSemaphore))
        ]

    d_inp = nc.declare_dram_parameter("inp", [128, C_END], f32, isOutput=False)
    d_o = nc.declare_dram_parameter("o", [128, 64], f32, isOutput=True)

    V, S, P, SY, G = nc.vector, nc.scalar, nc.tensor, nc.sync, nc.gpsimd

    def sb(name, w, dt=f32):
        return nc.alloc_sbuf_tensor(name, [128, w], dt)

    inp = sb("inpt", C_END)
    stim = inp[:, C_STIM:C_STIM + 8]
    gxe = inp[:, C_GXE:C_GXE + 8]
    gye = inp[:, C_GYE:C_GYE + 8]
    gxer = inp[:, C_GXER:C_GXER + 8]
    gyer = inp[:, C_GYER:C_GYER + 8]
    c_rheo = inp[:, C_CN:C_CN + 1]
    c_mkb = inp[:, C_CN + 1:C_CN + 2]
    c_zero = inp[:, C_CN + 2:C_CN + 3]
    c_one = inp[:, C_CN + 3:C_CN + 4]
    kv = inp[:, C_KV:C_KV + 16]
    pxs = inp[:, C_PXS:C_PXS + 64]
    pys = inp[:, C_PYS:C_PYS + 128]

    def ppc(i):
        return inp[:, C_PP + i:C_PP + i + 1]

    irho = sb("irho", 1)
    nxr = sb("nxr", 8)
    t4 = sb("t4", 8)
    gxn = sb("gxn", 8)
    gyn = sb("gyn", 8)
    qa = sb("qa", 8)
    y2 = sb("y2", 8)
    pab = sb("pab", 16)
    pbd = sb("pbd", 16)
    t5u5 = sb("t5u5", 16)
    spco = sb("spco", 16)
    er = sb("er", 8)
    e2 = sb("e2", 8)
    ewc = sb("ewc", 16)
    numewi = sb("numewi", 16)
    nb = sb("nb", 8)
    db = sb("db", 8)
    den = sb("den", 8)
    iden = sb("iden", 8)
    z0 = sb("z0", 16)
    vxy = sb("vxy", 16)     # [vxpx | vypx]
    q67 = sb("q67", 16)
    pk = sb("pk", 16)
    lnp = sb("lnp", 16)
    rsb = sb("rsb", 16)
    mk = sb("mk", 8)
    uu = sb("uu", 8)
    vv = sb("vv", 8)
    sgp = sb("sgp", 8)
    sg2 = sb("sg2", 8)
    bdt = bf16 if BF16_BULK else f32
    rs2 = sb("rs2", 8, bdt)   # 1/(2 sigma_px^2)
    tie = sb("tie", 8)
    ie = sb("ie", 8)
    exm = sb("exm", 8)
    u1 = sb("u1", 8)
    bamp = sb("bamp", 8, bf16)
    dxr = sb("dxr", 512, bdt)
    dyr = sb("dyr", 1024, bdt)
    dxr2 = sb("dxr2", 512, bdt)
    dyr2 = sb("dyr2", 1024, bdt)
    sq = sb("sq", 1536)
    g16 = sb("g16", 1536, bf16)
    gxb = sb("gxb", 512, bf16)
    o2 = sb("o2", 64)
    e1 = sb("e1", 64)
    e2p = sb("e2p", 64)
    tp = sb("tp", 64)
    t2 = sb("t2", 64)
    e3 = sb("e3", 64)
    ot = sb("ot", 64)
    ob = sb("ob", 64)
    acc = nc.alloc_psum_tensor("accp", [128, 64], f32)

    s_dma = nc.alloc_semaphore("s_dma")
    s_dm2 = nc.alloc_semaphore("s_dm2")
    s_dve = nc.alloc_semaphore("s_dve")
    s_act = nc.alloc_semaphore("s_act")
    s_pe = nc.alloc_semaphore("s_pe")
    s_gp = nc.alloc_semaphore("s_gp")
    nc._act_gate = (s_dma, 1)

    def bc(apx, dims):
        return AP(apx.tensor, apx.offset, [list(apx.ap[0])] + dims)

    nd = [0]
    na = [0]
    wt: dict = {}

    def _nm(x):
        try:
            return x.tensor.name
        except AttributeError:
            return None

    def dve(inst, outs, ins):
        if self_waits and nd[0] > 0:
            inst._wait_ge(s_dve, nd[0])
        else:
            need = 0
            for x in ins:
                nm = _nm(x)
                if nm is not None:
                    need = max(need, wt.get(nm, 0))
            if need > 0 and nd[0] - need < 8:
                inst._wait_ge(s_dve, need)
        inst.then_inc(s_dve, 1)
        nd[0] += 1
        for x in outs:
            nm = _nm(x)
            if nm is not None:
                wt[nm] = nd[0]
        return nd[0]

    def acti(inst):
        if self_waits and na[0] > 0:
            inst._wait_ge(s_act, na[0])
        inst.then_inc(s_act, 1)
        na[0] += 1
        return na[0]

    def ts(out, in0, s1, s2, op0, op1=None):
        if op1 is None:
            inst = V.tensor_scalar(out, in0, s1, None, op0)
        else:
            inst = V.tensor_scalar(out, in0, s1, s2, op0, op1)
        return dve(inst, [out], [in0, s1, s2])

    def tt(out, in0, in1, op):
        return dve(V.tensor_tensor(out, in0, in1, op), [out], [in0, in1])

    def stt(out, in0, s, in1, op0, op1):
        return dve(
            V.scalar_tensor_tensor(out, in0, s, in1, op0, op1),
            [out], [in0, s, in1],
        )

    def rcp(out, in0):
        return dve(V.reciprocal(out, in0), [out], [in0])

    # ================= program =================
    SY.dma_start(out=inp[:, 0:C_PXS], in_=d_inp[:, 0:C_PXS]).then_inc(
        s_dma, 16)
    SY.dma_start(out=inp[:, C_PXS:C_END], in_=d_inp[:, C_PXS:C_END]).then_inc(
        s_dm2, 16)

    # ---- DVE: rotation (ct ~= 1: |theta| <= 0.0175 rad) ----
    V.wait_ge(s_dma, 16)
    # nxr = gye*th - gxe (th = pp12*DEG2RAD folded into gyer)
    stt(nxr[:], gyer, ppc(12), gxe, OP.mult, OP.subtract)
    stt(t4[:], gxer, ppc(12), gye, OP.mult, OP.add)
    rcp(irho[:, 0:1], ppc(0))
    m_gxn = stt(gxn[:], bc(ppc(10), [[0, 8]]), 1.0 / 300.0, nxr[:],
                OP.mult, OP.subtract)
    ts(pk[:, 8:16], stim, irho[:, 0:1], 8e-05, OP.mult, OP.mult)
    m_gyn = stt(gyn[:], bc(ppc(11), [[0, 8]]), 1.0 / 300.0, t4[:],
                OP.mult, OP.add)

    # ---- ACT: er/qa on the critical path first, then the bamp chain ----
    S.wait_ge(s_dve, m_gxn)
    bz = c_zero if ZERO_BIAS_COLS else 0.0
    m_er = acti(S.activation(er[:], gxn[:], AF.Exp, scale=INVK, bias=bz))
    S.wait_ge(s_dve, m_gyn)
    m_qa = acti(S.activation(qa[:], gyn[:], AF.Square, scale=INVK,
                             bias=bz))

    # ---- DVE: sin/cos Estrin ----
    # I_eff relu chain on DVE (fills the wait for ACT's qa)
    ts(tie[:], stim, 8e-05, -RHEO, OP.mult, OP.add)
    m_ie = ts(ie[:], tie[:], 0.0, None, OP.max)
    S.wait_ge(s_dve, m_ie)
    acti(S.activation(exm[:], ie[:], AF.Exp, scale=-SLP, bias=bz))
    m_u1 = acti(S.activation(u1[:], exm[:], AF.Identity, scale=ESH,
                             bias=(c_one if ZERO_BIAS_COLS else 1.0)))
    # sin/cos Estrin + z chain, ordered so each op's DVE producer is a few
    # slots back (independent chains pad the serial hops)
    V.wait_ge(s_act, m_qa)
    tt(y2[:], qa[:], qa[:], OP.mult)
    ts(pbd[:, 0:8], qa[:], SIN_CK[3], SIN_CK[2], OP.mult, OP.add)
    ts(pbd[:, 8:16], qa[:], COS_C[3], COS_C[2], OP.mult, OP.add)
    V.wait_ge(s_act, m_er)
    tt(e2[:], er[:], er[:], OP.mult)
    ts(pab[:, 0:8], qa[:], SIN_CK[1], SIN_CK[0], OP.mult, OP.add)
    ts(pab[:, 8:16], qa[:], COS_C[1], COS_C[0], OP.mult, OP.add)
    tt(bc(t5u5[:, 0:16], [[8, 2], [1, 8]]),
       bc(y2[:, 0:8], [[0, 2], [1, 8]]),
       bc(pbd[:, 0:16], [[8, 2], [1, 8]]), OP.mult)
    ts(nb[:], e2[:], -A_, -B_, OP.mult, OP.add)
    ts(db[:], e2[:], A_ * A_, B_ * B_, OP.mult, OP.add)
    tt(spco[:], pab[:], t5u5[:], OP.add)
    V.wait_ge(s_act, m_u1)
    with nc.allow_low_precision("bamp feeds a bf16 matmul"):
        rcp(bamp[:], u1[:])
    tt(bc(ewc[:, 0:16], [[8, 2], [1, 8]]),
       bc(er[:, 0:8], [[0, 2], [1, 8]]),
       bc(spco[:, 0:16], [[8, 2], [1, 8]]), OP.mult)
    tt(numewi[:, 8:16], ewc[:, 0:8], gyn[:], OP.mult)
    stt(numewi[:, 0:8], ewc[:, 8:16], A_ + B_, nb[:], OP.mult, OP.add)
    stt(den[:], ewc[:, 8:16], -2.0 * AB, db[:], OP.mult, OP.add)
    rcp(iden[:], den[:])
    tt(bc(z0[:, 0:16], [[8, 2], [1, 8]]),
       bc(numewi[:, 0:16], [[8, 2], [1, 8]]),
       bc(iden[:, 0:8], [[0, 2], [1, 8]]), OP.mult)
    tt(q67[:], z0[:], z0[:], OP.mult)
    # [vxpx | vypx] = z0 * [C_VX.. | C_VY..]
    tt(vxy[:], z0[:], kv, OP.mult)
    m_pk = stt(pk[:, 0:8], q67[:, 8:16], BMA * BMA, q67[:, 0:8],
               OP.mult, OP.add)

    # ---- batched center offsets: x + y-lo on DVE, y-hi on GpSimd.
    # The GpSimd op waits for dyrA: DVE and GPSIMD share SBUF ports, so
    # running both big ops concurrently halves each one's bandwidth. ----
    V.wait_ge(s_dm2, 16)
    m_dxr = tt(bc(dxr[:, 0:512], [[64, 8], [1, 64]]),
               bc(pxs, [[0, 8], [1, 64]]),
               bc(vxy[:, 0:8], [[1, 8], [0, 64]]), OP.subtract)
    m_dyrA = tt(bc(dyr[:, 0:512], [[128, 4], [1, 128]]),
                bc(pys, [[0, 4], [1, 128]]),
                bc(vxy[:, 8:12], [[1, 4], [0, 128]]), OP.subtract)
    G.wait_ge(s_dm2, 16)
    G.wait_ge(s_dve, m_dyrA)
    G.tensor_tensor(
        bc(dyr[:, 512:1024], [[128, 4], [1, 128]]),
        bc(pys, [[0, 4], [1, 128]]),
        bc(vxy[:, 12:16], [[1, 4], [0, 128]]), OP.subtract,
    ).then_inc(s_gp, 1)

    # ---- ACT: packed sqrt [rr | sbase], mk, then the squares (frees DVE
    # for the sigma tail + sq scaling) ----
    S.wait_ge(s_dve, m_pk)
    acti(S.activation(lnp[:], pk[:], AF.Ln, bias=bz))
    acti(S.activation(rsb[:], lnp[:], AF.Exp, scale=0.5, bias=bz))
    m_mk = acti(S.activation(mk[:], rsb[:, 0:8], AF.Identity,
                             scale=CMA * (A_ + B_) * AB, bias=c_mkb))
    S.wait_ge(s_dve, m_dxr)
    acti(S.activation(dxr2[:], dxr[:], AF.Square, bias=bz))
    S.wait_ge(s_dve, m_dyrA)
    m_sqA = acti(S.activation(dyr2[:, 0:512], dyr[:, 0:512], AF.Square,
                              bias=bz))
    S.wait_ge(s_gp, 1)
    m_sqB = acti(S.activation(dyr2[:, 512:1024], dyr[:, 512:1024], AF.Square,
                              bias=bz))

    # ---- DVE: sigma -> rs2 = 1/(2 sigma_px^2) (bf16) ----
    V.wait_ge(s_act, m_mk)
    stt(uu[:], pk[:, 0:8], CMA * AB * AB, mk[:], OP.mult, OP.add)
    tt(vv[:], rsb[:, 8:16], uu[:], OP.mult)
    ts(sgp[:], vv[:], CSG * SQRT2, 0.5 * SQRT2, OP.mult, OP.max)
    tt(sg2[:], sgp[:], sgp[:], OP.mult)
    with nc.allow_low_precision("rs2 feeds bf16 gauss"):
        rcp(rs2[:], sg2[:])

    # ---- loop: sq pairs -> exp pairs (bf16) -> gxb -> matmul ----
    NPAIR = NCHUNK // 2
    m_sq = [0] * NPAIR
    m_exp = [0] * NPAIR
    m_gxb = [0] * NPAIR
    for p in range(NPAIR):
        V.wait_ge(s_act, m_sqA if p < 2 else m_sqB)
        tt(bc(sq[:, 384 * p:384 * p + 1], [[192, 2], [1, 64]]),
           bc(dxr2[:, 128 * p:128 * p + 1], [[64, 2], [1, 64]]),
           bc(rs2[:, 2 * p:2 * p + 2], [[1, 2], [0, 64]]), OP.mult)
        m_sq[p] = tt(
            bc(sq[:, 384 * p + 64:384 * p + 65], [[192, 2], [1, 128]]),
            bc(dyr2[:, 256 * p:256 * p + 1], [[128, 2], [1, 128]]),
            bc(rs2[:, 2 * p:2 * p + 2], [[1, 2], [0, 128]]), OP.mult)

    for p in range(NPAIR):
        S.wait_ge(s_dve, m_sq[p])
        m_exp[p] = acti(S.activation(g16[:, 384 * p:384 * p + 384],
                                     sq[:, 384 * p:384 * p + 384],
                                     AF.Exp, scale=-1.0, bias=bz))

    for p in range(NPAIR):
        V.wait_ge(s_act, m_exp[p])
        m_gxb[p] = tt(
            bc(gxb[:, 128 * p:128 * p + 1], [[64, 2], [1, 64]]),
            bc(g16[:, 384 * p:384 * p + 1], [[192, 2], [1, 64]]),
            bc(bamp[:, 2 * p:2 * p + 2], [[1, 2], [0, 64]]), OP.mult)

    for j in range(NCHUNK):
        p = j // 2
        P.wait_ge(s_dve, m_gxb[p])
        P.matmul(acc[:], g16[:, 192 * j + 64:192 * j + 192],
                 gxb[:, 64 * j:64 * j + 64],
                 start=(j == 0), stop=(j == NCHUNK - 1)).then_inc(s_pe, 1)

    # ---- tail: quartic polynomial; e1 on DVE, o2/e2p on ACT, from PSUM ----
    S.wait_ge(s_pe, NCHUNK)
    m_e2p = acti(S.activation(e2p[:], acc[:], AF.Identity, scale=ppc(6),
                              bias=ppc(5)))
    m_e1 = acti(S.activation(e1[:], acc[:], AF.Identity, scale=ppc(4),
                             bias=ppc(3)))
    # DVE cannot feed tensor_scalar straight from PSUM (hangs the NEFF);
    # copy through SBUF first.
    V.wait_ge(s_pe, NCHUNK)
    dve(V.tensor_copy(ot[:], acc[:]), [ot[:]], [acc[:]])
    tt(o2[:], ot[:], ot[:], OP.mult)
    V.wait_ge(s_act, m_e2p)
    stt(tp[:], o2[:], ppc(7), e2p[:], OP.mult, OP.add)
    tt(t2[:], o2[:], tp[:], OP.mult)
    V.wait_ge(s_act, m_e1)
    tt(e3[:], e1[:], t2[:], OP.add)
    m_ob = ts(ob[:], e3[:], 0.0, 1.0, OP.max, OP.min)

    SY.wait_ge(s_dve, m_ob)
    # the inc is consumed by nobody at runtime: the NEFF postamble's DRAIN +
    # full-range sem reset restores state for re-execution
    SY.dma_start(out=d_o[:], in_=ob[:]).then_inc(s_dma, 16)

    nc.finalize()
    _CACHE[key] = nc
    return nc


def _prep_in_maps(stim_np: np.ndarray, pp_np: np.ndarray):
    gxe, gye, xs = _host_constants()
    inp_base = np.empty((128, C_END), dtype=np.float32)
    inp_base[:, C_STIM:C_STIM + 8] = (
        stim_np.reshape(-1).astype(np.float32).reshape(NCHUNK, 128).T
    )
    inp_base[:, C_PP:C_PP + 13] = pp_np.reshape(1, 13).astype(np.float32)
    inp_base[:, C_GXE:C_GXE + 8] = gxe
    inp_base[:, C_GYE:C_GYE + 8] = gye
    inp_base[:, C_GXER:C_GXER + 8] = gxe * DEG2RAD
    inp_base[:, C_GYER:C_GYER + 8] = gye * DEG2RAD
    inp_base[:, C_CN] = -RHEO
    inp_base[:, C_CN + 1] = CMA * AB
    inp_base[:, C_CN + 2] = 0.0
    inp_base[:, C_CN + 3] = 1.0
    inp_base[:, C_KV:C_KV + 8] = C_VX
    inp_base[:, C_KV + 8:C_KV + 16] = C_VY
    in_maps = []
    for c in range(N_CORES):
        hh, wq = c // 4, c % 4
        inp = inp_base.copy()
        inp[:, C_PXS:C_PXS + 64] = xs[64 * wq:64 * wq + 64][None, :] * DEG2PIX
        inp[:, C_PYS:C_PYS + 128] = (
            xs[128 * hh:128 * hh + 128][None, :] * DEG2PIX
        )
        in_maps.append({"inp": inp})
    return in_maps


def _assemble(results) -> np.ndarray:
    out = np.empty((OUT, OUT), dtype=np.float32)
    for c in range(N_CORES):
        hh, wq = c // 4, c % 4
        out[128 * hh:128 * hh + 128, 64 * wq:64 * wq + 64] = results[c]["o"]
    return out.reshape(1, 1, OUT, OUT)


def kernel(stimulation: np.ndarray, patient_params: np.ndarray) -> np.ndarray:
    from concourse.bass_utils import run_bass_kernel_spmd

    stim_np = np.asarray(stimulation, dtype=np.float32)
    pp_np = np.asarray(patient_params, dtype=np.float32)
    nc = _build_nc()
    in_maps = _prep_in_maps(stim_np, pp_np)
    try:
        res = run_bass_kernel_spmd(nc, in_maps, list(range(N_CORES)))
    except Exception:
        res = run_bass_kernel_spmd(nc, in_maps, list(range(N_CORES)))
    return _assemble(res.results)
